# revision 7
# baseline (speedup 1.0000x reference)
"""Distributed Trainium2 kernel for a causal attention layer.

Reference computation (B=4, S=2048, D=1024, H=16, HD=64), f32:
    qkv = X @ w_attn + b_attn ; split to per-head q,k,v
    scores = (q @ k^T) masked causally (-1e5 pre-scale), / sqrt(64)
    attn = softmax(scores); ctx = attn @ v (heads merged)
    out = ctx @ w_proj + b_proj

Sharding (8 cores, tensor-parallel over heads):
  - Each core computes q,k,v for its 2 heads (w_attn column shard) for all
    tokens, runs causal attention for its (batch, head) pairs, and produces
    ctx^T [128 = 2*64 head-dims, tokens].
  - Per batch, ctx^T shards are AllGather'd (concat on partition axis ->
    full ctx^T [1024, 2048]); every core then computes a 128-wide output
    *column* slice of the projection (w_proj column shard) for all tokens —
    identical graph on every core, per-core behavior differs only via input
    data.
  - Host re-assembles output columns, adds b_proj.

All matmuls keep the contraction dim on partitions; the only on-chip
transposes are 128x128 PE transposes of v and ctx tiles. Causal block
sparsity skips fully-masked k-tiles; softmax uses the exp-then-normalize
form with the row-sum computed by an appended ones-column in v.
"""

import sys

sys.path.insert(0, "/opt/trn_rl_repo")

import numpy as np

import concourse.bass as bass
import concourse.bacc as bacc
import concourse.mybir as mybir
import concourse.tile as tile
from concourse.bass_utils import run_bass_kernel_spmd

B, S, D, H = 4, 2048, 1024, 16
HD = D // H  # 64
T = B * S  # 8192 tokens
TB = S  # tokens per batch
NC = 8  # cores
HLOC = H // NC  # 2 heads per core
QC = 512  # q-chunk (moving free dim for scores)
KT = 128  # k tile
F32 = mybir.dt.float32
F32R = mybir.dt.float32r

# matmul compute dtype: float32 (exact) or float32r (4x faster PE, slightly
# reduced precision). Flip MM_FAST to False to fall back to plain fp32.
MM_FAST = False


def _mm(ap):
    """Bitcast an f32 AP to float32r for fast PE mode."""
    if MM_FAST:
        return ap.bitcast(F32R)
    return ap


def build_nc():
    nc = bacc.Bacc("TRN2", target_bir_lowering=False, debug=False, num_devices=NC)

    xT = nc.declare_dram_parameter("xT", [D, T], F32, isOutput=False)
    wqkv = nc.declare_dram_parameter("wqkv", [D, 3 * 128], F32, isOutput=False)
    bias = nc.declare_dram_parameter("bias", [128, 3], F32, isOutput=False)
    masks = nc.declare_dram_parameter("masks", [4, 128, QC], F32, isOutput=False)
    ident = nc.declare_dram_parameter("ident", [128, 128], F32, isOutput=False)
    wproj = nc.declare_dram_parameter("wproj", [D, 128], F32, isOutput=False)
    outT = nc.declare_dram_parameter("outT", [128, T], F32, isOutput=True)

    # Internal DRAM bounce buffers for the per-batch AllGather of ctx^T.
    agin = [nc.dram_tensor(f"agin{b}", [128, TB], F32) for b in range(B)]
    agout = [
        nc.dram_tensor(f"agout{b}", [D, TB], F32, addr_space="Shared")
        for b in range(B)
    ]

    Exp = mybir.ActivationFunctionType.Exp
    Ident = mybir.ActivationFunctionType.Identity
    mult = mybir.AluOpType.mult

    with tile.TileContext(nc) as tc:
        with (
            tc.tile_pool(name="const", bufs=1) as cpool,
            tc.tile_pool(name="xt", bufs=10) as xtpool,
            tc.tile_pool(name="qkv", bufs=2) as qkvpool,
            tc.tile_pool(name="expt", bufs=18) as exptpool,
            tc.tile_pool(name="ctxs", bufs=8) as ctxspool,
            tc.tile_pool(name="ctxT", bufs=2) as ctxTpool,
            tc.tile_pool(name="ag", bufs=10) as agpool,
            tc.tile_pool(name="outsb", bufs=2) as outpool,
            tc.tile_pool(name="small", bufs=8) as smallpool,
            tc.tile_pool(name="pacc", bufs=2, space="PSUM") as pacc,
            tc.tile_pool(name="pscore", bufs=2, space="PSUM") as pscore,
            tc.tile_pool(name="pctx", bufs=2, space="PSUM") as pctx,
            tc.tile_pool(name="ptp", bufs=2, space="PSUM") as ptp,
        ):
            # ---- constants ----
            wqkv_sb = cpool.tile([128, 8 * 384], F32, name="wqkv_sb")
            for i in range(8):
                nc.sync.dma_start(
                    out=wqkv_sb[:, 384 * i : 384 * (i + 1)],
                    in_=wqkv[128 * i : 128 * (i + 1), :],
                )
            wp_sb = cpool.tile([128, 8 * 128], F32, name="wp_sb")
            for i in range(8):
                nc.sync.dma_start(
                    out=wp_sb[:, 128 * i : 128 * (i + 1)],
                    in_=wproj[128 * i : 128 * (i + 1), :],
                )
            bias_sb = cpool.tile([128, 3], F32, name="bias_sb")
            nc.sync.dma_start(out=bias_sb[:], in_=bias[:])
            ident_sb = cpool.tile([128, 128], F32, name="ident_sb")
            nc.sync.dma_start(out=ident_sb[:], in_=ident[:])
            mask_sb = []
            for m in range(4):
                mt = cpool.tile([128, QC], F32, name=f"mask{m}")
                nc.sync.dma_start(out=mt[:], in_=masks[m, :, :])
                mask_sb.append(mt)

            def emit_qkv(b):
                """QKV projection for batch b -> qT/kT [128, 2048], v [k,65|65] tiles."""
                qT = qkvpool.tile([128, TB], F32, name="qT", tag="qT")
                kT = qkvpool.tile([128, TB], F32, name="kT", tag="kT")
                # v with an appended ones column per head: per k-tile 130 cols
                v = qkvpool.tile([128, 16 * 130], F32, name="v", tag="v")
                nc.vector.memset(v[:], 1.0)
                for tci in range(TB // QC):  # 4 chunks of 512 tokens
                    xts = []
                    for i in range(8):
                        xt = xtpool.tile([128, QC], F32, name="xt", tag="xt")
                        nc.sync.dma_start(
                            out=xt[:],
                            in_=xT[
                                128 * i : 128 * (i + 1),
                                b * TB + tci * QC : b * TB + (tci + 1) * QC,
                            ],
                        )
                        xts.append(xt)
                    for which, dest, bcol in ((0, qT, 0), (1, kT, 1)):
                        ps = pacc.tile([128, QC], F32, name="qk_ps", tag="acc")
                        for i in range(8):
                            nc.tensor.matmul(
                                ps[:],
                                lhsT=_mm(
                                    wqkv_sb[:, 384 * i + 128 * which : 384 * i + 128 * which + 128]
                                ),
                                rhs=_mm(xts[i][:]),
                                start=(i == 0),
                                stop=(i == 7),
                            )
                        nc.scalar.activation(
                            dest[:, tci * QC : (tci + 1) * QC],
                            ps[:],
                            Ident,
                            bias=bias_sb[:, bcol : bcol + 1],
                        )
                    # v^T then PE-transpose into natural [token, headdim] layout
                    ps = pacc.tile([128, QC], F32, name="v_ps", tag="acc")
                    for i in range(8):
                        nc.tensor.matmul(
                            ps[:],
                            lhsT=_mm(wqkv_sb[:, 384 * i + 256 : 384 * i + 384]),
                            rhs=_mm(xts[i][:]),
                            start=(i == 0),
                            stop=(i == 7),
                        )
                    vt = smallpool.tile([128, QC], F32, name="vt", tag="vt", bufs=2)
                    nc.scalar.activation(vt[:], ps[:], Ident, bias=bias_sb[:, 2:3])
                    for j in range(4):
                        kt = tci * 4 + j
                        tp = ptp.tile([128, 128], F32, name="vtp", tag="tp")
                        nc.tensor.transpose(tp[:], vt[:, 128 * j : 128 * (j + 1)], ident_sb[:])
                        dst = v[:, 130 * kt : 130 * kt + 130].rearrange(
                            "p (g c) -> p g c", c=65
                        )[:, :, 0:64]
                        src = tp[:].rearrange("p (g c) -> p g c", c=64)
                        nc.scalar.copy(out=dst, in_=src)
                return qT, kT, v

            def emit_attn(b, qT, kT, v):
                """Causal attention for batch b, both local heads -> ctxT strip."""
                ctxT = ctxTpool.tile([128, TB], F32, name="ctxT", tag="ctxT")
                for qc in range(TB // QC):
                    cbs = {}
                    for h in range(HLOC):
                        hp = 64 * h
                        n_kt = 4 * qc + 4
                        ets = []
                        for kt in range(n_kt):
                            sps = pscore.tile([128, QC], F32, name="s_ps", tag="score")
                            nc.tensor.matmul(
                                sps[:],
                                lhsT=_mm(kT[hp : hp + 64, KT * kt : KT * (kt + 1)]),
                                rhs=_mm(qT[hp : hp + 64, QC * qc : QC * (qc + 1)]),
                                start=True,
                                stop=True,
                            )
                            et = exptpool.tile([128, QC], F32, name="et", tag="expt")
                            nc.scalar.activation(et[:], sps[:], Exp, scale=0.125)
                            if kt >= 4 * qc:  # diagonal block: causal mask
                                nc.vector.tensor_tensor(
                                    et[:], et[:], mask_sb[kt - 4 * qc][:], mult
                                )
                            ets.append(et)
                        for j in range(4):
                            qt = 4 * qc + j
                            cps = pctx.tile([128, 128], F32, name="c_ps", tag="ctx")
                            for kt in range(qt + 1):
                                nc.tensor.matmul(
                                    cps[:, 0:65],
                                    lhsT=_mm(ets[kt][:, 128 * j : 128 * (j + 1)]),
                                    rhs=_mm(v[:, 130 * kt + 65 * h : 130 * kt + 65 * h + 65]),
                                    start=(kt == 0),
                                    stop=(kt == qt),
                                )
                            recip = smallpool.tile([128, 1], F32, name="recip", tag="recip")
                            nc.vector.reciprocal(recip[:], cps[:, 64:65])
                            if h == 0:
                                cb = ctxspool.tile([128, 128], F32, name="cb", tag="ctxs")
                                cbs[j] = cb
                            else:
                                cb = cbs[j]
                            nc.vector.tensor_scalar_mul(
                                cb[:, hp : hp + 64], cps[:, 0:64], recip[:]
                            )
                    for j in range(4):
                        qt = 4 * qc + j
                        tp = ptp.tile([128, 128], F32, name="ctp", tag="tp")
                        nc.tensor.transpose(tp[:], cbs[j][:], ident_sb[:])
                        nc.scalar.copy(
                            out=ctxT[:, 128 * qt : 128 * (qt + 1)], in_=tp[:]
                        )
                return ctxT

            def emit_proj(b):
                """Project batch b: outT[:, b*2048:+2048] = (ctx_b @ wproj_slice)^T."""
                osb = outpool.tile([128, TB], F32, name="osb", tag="osb")
                for tci in range(TB // QC):
                    ps = pacc.tile([128, QC], F32, name="p_ps", tag="acc")
                    for i in range(8):
                        agt = agpool.tile([128, QC], F32, name="agt", tag="ag")
                        nc.sync.dma_start(
                            out=agt[:],
                            in_=agout[b][
                                128 * i : 128 * (i + 1), tci * QC : (tci + 1) * QC
                            ],
                        )
                        nc.tensor.matmul(
                            ps[:],
                            lhsT=_mm(wp_sb[:, 128 * i : 128 * (i + 1)]),
                            rhs=_mm(agt[:]),
                            start=(i == 0),
                            stop=(i == 7),
                        )
                    nc.scalar.copy(out=osb[:, tci * QC : (tci + 1) * QC], in_=ps[:])
                nc.sync.dma_start(out=outT[:, b * TB : (b + 1) * TB], in_=osb[:])

            for b in range(B):
                qT, kT, v = emit_qkv(b)
                ctxT = emit_attn(b, qT, kT, v)
                nc.sync.dma_start(out=agin[b][:], in_=ctxT[:])
                nc.gpsimd.collective_compute(
                    "AllGather",
                    mybir.AluOpType.bypass,
                    replica_groups=[list(range(NC))],
                    ins=[agin[b][:]],
                    outs=[agout[b][:]],
                )
                if b >= 1:
                    emit_proj(b - 1)
            emit_proj(B - 1)

    nc.compile()
    return nc


_NC_CACHE = None


def _get_nc():
    global _NC_CACHE
    if _NC_CACHE is None:
        _NC_CACHE = build_nc()
    return _NC_CACHE


def make_in_maps(hidden_states, w_attn, b_attn, w_proj):
    x = np.ascontiguousarray(hidden_states.reshape(T, D).T)  # [D, T]
    masks = np.zeros((4, 128, QC), dtype=np.float32)
    for m in range(4):
        k = np.arange(128)[:, None] + 128 * m
        q = np.arange(QC)[None, :]
        masks[m] = (k <= q).astype(np.float32)
    ident = np.eye(128, dtype=np.float32)
    in_maps = []
    for c in range(NC):
        lo = 128 * c
        wqkv = np.concatenate(
            [
                w_attn[:, lo : lo + 128],
                w_attn[:, D + lo : D + lo + 128],
                w_attn[:, 2 * D + lo : 2 * D + lo + 128],
            ],
            axis=1,
        ).astype(np.float32)
        bias = np.stack(
            [
                b_attn[lo : lo + 128],
                b_attn[D + lo : D + lo + 128],
                b_attn[2 * D + lo : 2 * D + lo + 128],
            ],
            axis=1,
        ).astype(np.float32)
        in_maps.append(
            {
                "xT": x,
                "wqkv": np.ascontiguousarray(wqkv),
                "bias": np.ascontiguousarray(bias),
                "masks": masks,
                "ident": ident,
                "wproj": np.ascontiguousarray(w_proj[:, lo : lo + 128]).astype(
                    np.float32
                ),
            }
        )
    return in_maps


def run(inputs, trace=False):
    nc = _get_nc()
    in_maps = make_in_maps(
        np.asarray(inputs["hidden_states"]),
        np.asarray(inputs["w_attn"]),
        np.asarray(inputs["b_attn"]),
        np.asarray(inputs["w_proj"]),
    )
    res = run_bass_kernel_spmd(nc, in_maps, core_ids=list(range(NC)), trace=trace)
    out = np.empty((T, D), dtype=np.float32)
    for c in range(NC):
        out[:, 128 * c : 128 * (c + 1)] = res.results[c]["outT"].T
    out += np.asarray(inputs["b_proj"])[None, :].astype(np.float32)
    return out.reshape(B, S, D), res


def kernel(**inputs):
    out, _ = run(inputs)
    return out
